# revision 1
# baseline (speedup 1.0000x reference)
"""Neural CDE (RK4 3/8, 63 steps) Trainium2 Bass kernel.

Data-parallel over batch: B=1024 split across 8 NeuronCores (128/core).
All computation in f32 end-to-end — the CDE dynamics amplify per-step
rounding noise ~1000x, so 16-bit operands anywhere push the final error
to ~0.3 relative; f32 lands at the reference's own rounding floor (~6e-4).

Per core, everything is kept H-major ([H, B_local] tiles, H=128 partitions):

  per RK4 stage i (4 per step):
    zpre_i = W1 @ arg_i                                   (PE)
    zT     = tanh(zpre_i + b1)                            (ACT, PSUM->SBUF)
    zf_f   = zT * broadcast(dX_i[f, :])   f = 0..15       (DVE+GPSIMD)
    k_i    = sum_f A_f @ zf_f  +  b2r @ dX_iT             (PE, PSUM accumulate)
    k_i'   = dt * k_i                                     (ACT, PSUM->SBUF)
  arg_i / h update built on DVE via scalar_tensor_tensor chains.

A_f[j, h] = W2[h*F+f, j]: the einsum over F is folded into 16 PSUM-accumulated
matmuls. dX values for all (step, stage) are precomputed on host (cheap,
O(B*S*F) work) and broadcast to 128 partitions per stage by a DRAM->SBUF DMA
(overlapped with compute).
"""

import numpy as np
import sys

sys.path.insert(0, "/opt/trn_rl_repo")

H, F, B, S = 128, 16, 1024, 64
NC = 8
BL = B // NC          # 128 batch per core
NS = S - 1            # 63 steps
NST = NS * 4          # 252 stages

_compiled = None

# KR product split: f-slices 0..KR_DVE-1 on VectorE, rest on GpSimd
KR_DVE = 11


def _host_prep(x, W1, b1, W2, b2, Wi, bi):
    """Host-side precompute: Hermite-cubic dX table + weight repacks."""
    f32 = np.float32
    x = np.asarray(x, f32)

    times = np.linspace(0.0, 1.0, S, dtype=f32)
    dt = (times[1:] - times[:-1]).astype(np.float64)            # [63]
    dtv = dt[None, :, None]
    xd = x.astype(np.float64)
    p0, p1 = xd[:, :-1], xd[:, 1:]
    seg = (p1 - p0) / dtv                                       # [B, 63, F]
    m0 = np.concatenate([seg[:, :1], seg[:, :-1]], axis=1)
    m1 = seg
    c = (3.0 * seg - (2.0 * m0 + m1)) / dtv
    d = (-2.0 * seg + (m0 + m1)) / (dtv * dtv)
    bco = m0

    # dX(f) = b + (2c + 3d*f)*f at f in {0, dt/3, 2dt/3, dt}
    dX = np.empty((NS, 4, B, F), np.float64)                    # [s, r, b, f]
    for r, frac in enumerate((0.0, 1.0 / 3.0, 2.0 / 3.0, 1.0)):
        fr = (dt * frac)[None, :, None]
        v = bco + (2.0 * c + 3.0 * d * fr) * fr                 # [B, 63, F]
        dX[:, r] = np.swapaxes(v, 0, 1)

    # per-core tables:
    # dxt[core][t, f, b]     (b2-matmul rhs, staged per stage; [NST, 16, BL])
    # rflat[core][t, f*BL+b] (R broadcast DMA source; [NST, F*BL])
    dxt = np.empty((NC, NST, F, BL), f32)
    rflat = np.empty((NC, NST, F * BL), f32)
    for core in range(NC):
        sl = dX[:, :, core * BL:(core + 1) * BL, :]             # [s, r, BL, F]
        t_fb = np.transpose(sl.reshape(NST, BL, F), (0, 2, 1))  # [t, F, BL]
        dxt[core] = t_fb.astype(f32)
        rflat[core] = t_fb.reshape(NST, F * BL).astype(f32)

    W1 = np.asarray(W1, f32); b1 = np.asarray(b1, f32)
    W2 = np.asarray(W2, f32); b2 = np.asarray(b2, f32)
    Wi = np.asarray(Wi, f32); bi = np.asarray(bi, f32)

    w1t = W1.T.astype(f32).copy()                                     # [128, 128]
    apack = np.concatenate(
        [W2[f::F, :].T for f in range(F)], axis=1).astype(f32)        # [128, 2048]
    b2rt = b2.reshape(H, F).T.astype(f32).copy()                      # [16, 128]
    b1c = b1.reshape(H, 1).astype(f32).copy()                         # [128, 1]
    wit = Wi.T.astype(f32).copy()                                     # [16, 128]
    birow = bi.reshape(1, H).astype(f32).copy()                       # [1, 128]
    ones = np.ones((1, BL), f32)

    x0t = np.empty((NC, F, BL), f32)
    for core in range(NC):
        x0t[core] = x[core * BL:(core + 1) * BL, 0, :].T

    in_maps = []
    for core in range(NC):
        in_maps.append({
            "dxt": np.ascontiguousarray(dxt[core]),
            "rflat": np.ascontiguousarray(rflat[core]),
            "w1t": w1t,
            "apack": apack,
            "b2rt": b2rt,
            "b1c": b1c,
            "wit": wit,
            "birow": birow,
            "ones": ones,
            "x0t": np.ascontiguousarray(x0t[core]),
        })
    return in_maps, dt.astype(f32)


def _build(dt_f32):
    """Build + compile the Bass/Tile kernel (shapes and dt are static)."""
    import concourse.bacc as bacc
    import concourse.mybir as mybir
    from concourse import tile

    f32 = mybir.dt.float32
    Tanh = mybir.ActivationFunctionType.Tanh
    Copy = mybir.ActivationFunctionType.Copy
    MUL = mybir.AluOpType.mult
    ADD = mybir.AluOpType.add

    nc = bacc.Bacc("TRN2", target_bir_lowering=False, debug=False)

    d_dxt = nc.dram_tensor("dxt", [NST, F, BL], f32, kind="ExternalInput")
    d_rflat = nc.dram_tensor("rflat", [NST, F * BL], f32, kind="ExternalInput")
    d_w1t = nc.dram_tensor("w1t", [H, H], f32, kind="ExternalInput")
    d_apack = nc.dram_tensor("apack", [H, F * H], f32, kind="ExternalInput")
    d_b2rt = nc.dram_tensor("b2rt", [F, H], f32, kind="ExternalInput")
    d_b1c = nc.dram_tensor("b1c", [H, 1], f32, kind="ExternalInput")
    d_wit = nc.dram_tensor("wit", [F, H], f32, kind="ExternalInput")
    d_birow = nc.dram_tensor("birow", [1, H], f32, kind="ExternalInput")
    d_ones = nc.dram_tensor("ones", [1, BL], f32, kind="ExternalInput")
    d_x0t = nc.dram_tensor("x0t", [F, BL], f32, kind="ExternalInput")
    d_hout = nc.dram_tensor("hout", [H, BL], f32, kind="ExternalOutput")

    with tile.TileContext(nc) as tc:
        with tc.tile_pool(name="const", bufs=1) as cpool, \
             tc.tile_pool(name="work", bufs=2) as wpool, \
             tc.tile_pool(name="kbuf", bufs=8) as kpool, \
             tc.tile_pool(name="rbuf", bufs=3) as rpool, \
             tc.tile_pool(name="dxbuf", bufs=3) as dxpool, \
             tc.tile_pool(name="zfbuf", bufs=2) as zfpool, \
             tc.tile_pool(name="psA", bufs=2, space="PSUM") as psA, \
             tc.tile_pool(name="psB", bufs=2, space="PSUM") as psB:

            # ---- load constants to SBUF ----
            sb_w1t = cpool.tile([H, H], f32, tag="w1t")
            sb_apack = cpool.tile([H, F * H], f32, tag="apack")
            sb_b2rt = cpool.tile([F, H], f32, tag="b2rt")
            sb_b1c = cpool.tile([H, 1], f32, tag="b1c")
            sb_wit = cpool.tile([F, H], f32, tag="wit")
            sb_birow = cpool.tile([1, H], f32, tag="birow")
            sb_ones = cpool.tile([1, BL], f32, tag="ones")
            sb_x0t = cpool.tile([F, BL], f32, tag="x0t")
            for sb, dr in ((sb_w1t, d_w1t), (sb_apack, d_apack),
                           (sb_b2rt, d_b2rt), (sb_b1c, d_b1c), (sb_wit, d_wit),
                           (sb_birow, d_birow), (sb_ones, d_ones), (sb_x0t, d_x0t)):
                nc.sync.dma_start(sb[:, :], dr.ap())

            # ---- h0 = Wi @ x0 + bi ----
            p0 = psA.tile([H, BL], f32, tag="zpre")
            nc.tensor.matmul(p0[:, :], sb_wit[:, :], sb_x0t[:, :],
                             start=True, stop=False)
            nc.tensor.matmul(p0[:, :], sb_birow[:, :], sb_ones[:, :],
                             start=False, stop=True)
            h32 = wpool.tile([H, BL], f32, tag="h32")
            nc.scalar.activation(h32[:, :], p0[:, :], Copy)

            ks = [None] * 4
            for s in range(NS):
                dts = float(dt_f32[s])
                args = [h32, None, None, None]
                for i in range(4):
                    t = 4 * s + i
                    # R broadcast: dX_i replicated over 128 partitions
                    R = rpool.tile([H, F * BL], f32, tag="R")
                    nc.sync.dma_start(
                        R[:, :], d_rflat.ap()[t:t + 1, :].partition_broadcast(H))
                    # staged dX slice (b2 matmul rhs)
                    dxs = dxpool.tile([F, BL], f32, tag="dxs")
                    nc.sync.dma_start(dxs[:, :], d_dxt.ap()[t, :, :])

                    # zpre = W1 @ arg_i
                    zp = psA.tile([H, BL], f32, tag="zpre")
                    nc.tensor.matmul(zp[:, :], sb_w1t[:, :], args[i][:, :],
                                     start=True, stop=True)

                    # zT = tanh(zpre + b1)  -> SBUF
                    zT = wpool.tile([H, BL], f32, tag="zT")
                    nc.scalar.activation(zT[:, :], zp[:, :], Tanh,
                                         bias=sb_b1c[:, :])

                    # KR product: zf[:, f*BL:(f+1)*BL] = zT * R_f
                    zf = zfpool.tile([H, F * BL], f32, tag="zf")
                    for f in range(F):
                        eng = nc.vector if f < KR_DVE else nc.gpsimd
                        fs = slice(f * BL, (f + 1) * BL)
                        eng.tensor_mul(zf[:, fs], zT[:, :], R[:, fs])

                    # k_i = b2r @ dX_iT + sum_f A_f @ zf_f
                    kp = psB.tile([H, BL], f32, tag="kp")
                    nc.tensor.matmul(kp[:, :], sb_b2rt[:, :], dxs[:, :],
                                     start=True, stop=False)
                    for f in range(F):
                        fs = slice(f * H, (f + 1) * H)
                        fb = slice(f * BL, (f + 1) * BL)
                        nc.tensor.matmul(kp[:, :], sb_apack[:, fs], zf[:, fb],
                                         start=False, stop=(f == F - 1))

                    # k_i' = dt * k_i -> SBUF
                    ki = kpool.tile([H, BL], f32, tag="k16", name=f"k16_{t}")
                    nc.scalar.activation(ki[:, :], kp[:, :], Copy, scale=dts)
                    ks[i] = ki

                    # build next arg on DVE:
                    #   a2 = h + (1/3)k1'
                    #   a3 = h + k2' - (1/3)k1'
                    #   a4 = h + k1' - k2' + k3'
                    if i == 0:
                        a2 = wpool.tile([H, BL], f32, tag="a2")
                        nc.vector.scalar_tensor_tensor(
                            a2[:, :], ks[0][:, :], 1.0 / 3.0, h32[:, :],
                            op0=MUL, op1=ADD)
                        args[1] = a2
                    elif i == 1:
                        t3 = wpool.tile([H, BL], f32, tag="t3")
                        a3 = wpool.tile([H, BL], f32, tag="a3")
                        nc.vector.scalar_tensor_tensor(
                            t3[:, :], ks[0][:, :], -1.0 / 3.0, h32[:, :],
                            op0=MUL, op1=ADD)
                        nc.vector.scalar_tensor_tensor(
                            a3[:, :], ks[1][:, :], 1.0, t3[:, :],
                            op0=MUL, op1=ADD)
                        args[2] = a3
                    elif i == 2:
                        t5 = wpool.tile([H, BL], f32, tag="t5")
                        t6 = wpool.tile([H, BL], f32, tag="t6")
                        a4 = wpool.tile([H, BL], f32, tag="a4")
                        nc.vector.scalar_tensor_tensor(
                            t5[:, :], ks[0][:, :], 1.0, h32[:, :],
                            op0=MUL, op1=ADD)
                        nc.vector.scalar_tensor_tensor(
                            t6[:, :], ks[1][:, :], -1.0, t5[:, :],
                            op0=MUL, op1=ADD)
                        nc.vector.scalar_tensor_tensor(
                            a4[:, :], ks[2][:, :], 1.0, t6[:, :],
                            op0=MUL, op1=ADD)
                        args[3] = a4

                # h += (k1' + 3 k2' + 3 k3' + k4')/8
                u1 = wpool.tile([H, BL], f32, tag="u1")
                u2 = wpool.tile([H, BL], f32, tag="u2")
                u3 = wpool.tile([H, BL], f32, tag="u3")
                h32n = wpool.tile([H, BL], f32, tag="h32")
                nc.vector.scalar_tensor_tensor(
                    u1[:, :], ks[0][:, :], 0.125, h32[:, :], op0=MUL, op1=ADD)
                nc.vector.scalar_tensor_tensor(
                    u2[:, :], ks[1][:, :], 0.375, u1[:, :], op0=MUL, op1=ADD)
                nc.vector.scalar_tensor_tensor(
                    u3[:, :], ks[2][:, :], 0.375, u2[:, :], op0=MUL, op1=ADD)
                nc.vector.scalar_tensor_tensor(
                    h32n[:, :], ks[3][:, :], 0.125, u3[:, :], op0=MUL, op1=ADD)
                h32 = h32n

            nc.sync.dma_start(d_hout.ap(), h32[:, :])

    nc.compile()
    return nc


def _get_compiled():
    global _compiled
    if _compiled is None:
        f32 = np.float32
        times = np.linspace(0.0, 1.0, S, dtype=f32)
        dt_f32 = times[1:] - times[:-1]
        _compiled = _build(dt_f32)
    return _compiled


def run(inputs, trace=False, trace_kwargs=None):
    """Returns (full_output [B, H] f32, BassKernelResults)."""
    from concourse import bass_utils

    nc = _get_compiled()
    in_maps, _ = _host_prep(**inputs)
    res = bass_utils.run_bass_kernel_spmd(
        nc, in_maps, core_ids=list(range(NC)), trace=trace,
        **(trace_kwargs or {}))
    out = np.concatenate(
        [res.results[c]["hout"].T for c in range(NC)], axis=0)
    return np.ascontiguousarray(out.astype(np.float32)), res


def kernel(**inputs):
    out, _ = run(inputs)
    return out

